# revision 11
# baseline (speedup 1.0000x reference)
"""MultiHeadedAttention Trainium2 kernel (8 NeuronCores).

Sharding: core c -> (batch b = c//2, head-group g = c%2). Each core computes
the 8-head attention slice for one batch plus its partial output projection;
the host sums the two partials per batch and adds the output bias.

Device-side layout is feature-major: the host ships q/k/v pre-transposed
([E, L], bf16) so every matmul contracts along SBUF partitions without any
on-chip transpose. The attention scale 1/sqrt(hd) is folded into Wq/bq on
the host. Projection biases are applied during the PSUM->SBUF cast via DVE
tensor_scalar (per-partition bias AP for q/k; broadcast row for v).

The kernel is paced by PE matmul columns and the ScalarE exp of the 256
score tiles ([128,1024] each, ~1.15us apiece), which are roughly
balanced; everything else is scheduled to hide under that wall:
  - dummy warmup matmuls + single-descriptor [128,4096] DMAs keep the PE
    HAM clock-gate warm from t~1us (2.4GHz instead of 1.2).
  - softmax denominators ride the PV matmuls for free: each head's V
    slice carries a ones column (per-head width 65), so the M=65 PV
    accumulation leaves sum_k exp(S) in PSUM row 64. This removes the
    512 M=1 ones-matmuls (~109us of PE time) the old design spent on
    denominators. Normalization: reciprocal_approx_fast + gpsimd
    partition_broadcast + one DVE multiply per (head, q-block).
  - q/k/v projections interleave into the early steps' PE slack;
    the output projection of q-block 0 overlaps the last step.
"""

import math
import sys

sys.path.insert(0, "/opt/trn_rl_repo")

import numpy as np
import ml_dtypes

import concourse.bass as bass  # noqa: F401  (registers rust bindings)
import concourse.mybir as mybir
import concourse.tile as tile
from concourse import bacc
from concourse.bass_utils import run_bass_kernel_spmd

BF16_NP = ml_dtypes.bfloat16
F32 = mybir.dt.float32
BF16 = mybir.dt.bfloat16

B, L, E, H, HD = 4, 2048, 1024, 16, 64
NCORES = 8
D = 512          # per-core projection width (8 heads * 64)
P = 128
ET = E // P      # 8 contraction tiles over E
PT = D // P      # 4 partition-tiles of qpT/kpT
TT = L // P      # 16 token tiles
QW = 512         # matmul moving free width

WARMUP_MMS = 72
EXPS_BUFS = 33

TRACE = False
LAST_EXEC_NS = None
LAST_RESULTS = None

# step order: (head-pair, q-block). Pair 3 blk 0 runs at idx 4 so its PV
# (idx 5) and norm (idx 6 g2-9) finish early enough for the blk-0 output
# projection to ride the idle PE slack of steps 6-7.
STEPS = [(0, 0), (0, 1), (1, 0), (2, 0), (3, 0), (1, 1), (2, 1), (3, 1)]

Add = mybir.AluOpType.add


def _emit(nc, tc, io):
    Exp = mybir.ActivationFunctionType.Exp
    qT, kT, vT = io["qT"], io["kT"], io["vT"]
    wq_d, wk_d, wv_d, wo_d = io["wq"], io["wk"], io["wv"], io["wo"]
    bq_d, bk_d, bv_d = io["bq"], io["bk"], io["bv"]
    out = io["out"]

    import contextlib
    stack = contextlib.ExitStack()
    with stack:
        pers = stack.enter_context(tc.tile_pool(name="pers", bufs=1))
        inx = stack.enter_context(tc.tile_pool(name="inx", bufs=3))
        expS = stack.enter_context(tc.tile_pool(name="expS", bufs=EXPS_BUFS))
        ps_pool = stack.enter_context(
            tc.tile_pool(name="ps", bufs=2, space="PSUM"))
        pv_pool = stack.enter_context(
            tc.tile_pool(name="pv", bufs=4, space="PSUM"))
        ost_pool = stack.enter_context(tc.tile_pool(name="ost", bufs=2))
        scs_pool = stack.enter_context(tc.tile_pool(name="scs", bufs=4))
        bc_pool = stack.enter_context(tc.tile_pool(name="bc", bufs=1))

        # ---- persistent SBUF ----
        qpT = [pers.tile([P, L], BF16, tag=f"qpT{i}", name=f"qpT{i}")
               for i in range(PT)]
        kpT = [pers.tile([P, L], BF16, tag=f"kpT{i}", name=f"kpT{i}")
               for i in range(PT)]
        OT = [pers.tile([P, L], BF16, tag=f"OT{i}", name=f"OT{i}")
              for i in range(PT)]
        # per-head width 65: cols 65h..65h+63 = V dims, col 65h+64 = 1.0
        # (the ones column makes the M=65 PV matmul accumulate the softmax
        # denominator in PSUM row 64)
        vpa = [pers.tile([P, H // 2 * 65], BF16, tag=f"vpa{t}", name=f"vpa{t}")
               for t in range(TT)]
        w_sb = {nm: pers.tile([P, ET * D], BF16, tag=f"w{nm}", name=f"w{nm}")
                for nm in ("q", "k", "v")}
        wo_sb = pers.tile([P, 4 * E], BF16, tag="wo", name="wo")
        bqk = {nm: pers.tile([P, PT], F32, tag=f"b{nm}", name=f"b{nm}")
               for nm in ("q", "k")}
        bv_row = pers.tile([1, D], F32, tag="bvr", name="bvr")
        bv_b = pers.tile([P, D], F32, tag="bvb", name="bvb")
        scratch = pers.tile([P, 256], BF16, tag="scr", name="scr")

        # ---- warmup: keep the PE busy (and the HAM clock-gate warming)
        # while the first weight/input DMAs land ----
        nc.vector.memset(scratch[:], 0.125)
        for t in range(TT):
            nc.vector.memset(
                vpa[t].rearrange("p (h c) -> p h c", c=65)[:, :, 64:65], 1.0)
        def dummy_mms(n, tile=None):
            wt = tile if tile is not None else ps_pool.tile(
                [P, QW], F32, tag="s", name="warm")
            for _ in range(n):
                nc.tensor.matmul(wt[:, 0:256], scratch[:, 0:P],
                                 scratch[:], start=True, stop=True)

        # ---- weight loads ----
        def w_view(wdram):
            return wdram.rearrange("(e p) d -> p e d", p=P)

        def dma_w_half(nm, wdram, h, eng):
            eng.dma_start(
                out=w_sb[nm][:, 4 * D * h:4 * D * (h + 1)].rearrange(
                    "p (e d) -> p e d", d=D),
                in_=wdram[E // 2 * h:E // 2 * (h + 1), :].rearrange(
                    "(e p) d -> p e d", p=P))

        def dma_x_half(xT, qu, xt, h, eng):
            eng.dma_start(
                out=xt[:, 4 * QW * h:4 * QW * (h + 1)].rearrange(
                    "p (e t) -> p e t", t=QW),
                in_=xT[E // 2 * h:E // 2 * (h + 1),
                       QW * qu:QW * (qu + 1)].rearrange(
                    "(e p) t -> p e t", p=P))

        def late_loads():
            nc.gpsimd.dma_start(
                out=w_sb["v"].rearrange("p (e d) -> p e d", d=D),
                in_=w_view(wv_d))
            nc.gpsimd.dma_start(out=bv_row[:], in_=bv_d)
            nc.gpsimd.partition_broadcast(bv_b[:], bv_row[:], channels=P)
            nc.gpsimd.dma_start(
                out=wo_sb.rearrange("p (c e) -> p c e", e=E),
                in_=wo_d.rearrange("(c p) e -> p c e", p=P))

        # ---- projection building blocks ----
        # Per-queue DMA bandwidth is only ~130 GB/s, so spread the 1MB
        # quarter loads across three issue queues (scalar is reserved: an
        # exp stalled behind a DMA issue costs wall time directly).
        dma_flip = [0]
        _qrot = (nc.sync, nc.gpsimd)

        def dma_quarter(xT, qu, eng=None):
            xt = inx.tile([P, ET * QW], BF16, tag="inx", name="inx")
            if eng is None:
                eng = _qrot[dma_flip[0] % 2]
                dma_flip[0] += 1
            eng.dma_start(
                out=xt.rearrange("p (e t) -> p e t", t=QW),
                in_=xT[:, QW * qu:QW * (qu + 1)].rearrange(
                    "(e p) t -> p e t", p=P))
            return xt

        def qk_group(nm, dst, xt, qu, i):
            """One psum group: qpT/kpT pd-tile i, token quarter qu."""
            ps = ps_pool.tile([P, QW], F32, tag="s", name="ps")
            for e in range(ET):
                nc.tensor.matmul(
                    ps[:], w_sb[nm][:, D * e + P * i:D * e + P * (i + 1)],
                    xt[:, QW * e:QW * (e + 1)],
                    start=(e == 0), stop=(e == ET - 1))
            nc.vector.tensor_scalar(
                dst[i][:, QW * qu:QW * (qu + 1)], ps[:],
                bqk[nm][:, i:i + 1], None, Add)

        def v_group(xt, qu, tt_):
            t = 4 * qu + tt_
            ps = ps_pool.tile([P, D], F32, tag="s", name="ps")
            for e in range(ET):
                nc.tensor.matmul(
                    ps[:], xt[:, QW * e + P * tt_:QW * e + P * (tt_ + 1)],
                    w_sb["v"][:, D * e:D * (e + 1)],
                    start=(e == 0), stop=(e == ET - 1))
            nc.vector.tensor_tensor(
                vpa[t].rearrange("p (h c) -> p h c", c=65)[:, :, 0:64],
                ps[:].rearrange("p (h c) -> p h c", c=64),
                bv_b[:].rearrange("p (h c) -> p h c", c=64), Add)

        # Interleaved projection task lists, one per early attention step.
        # Tokens: "D:nm:qu" = quarter DMA, "G:nm:qu:i" = qk group,
        # "V:qu:tt" = v group. Quarter DMAs sit ~6 task slots ahead of
        # their consumer group (a 1MB transfer takes ~7us on contended HBM).
        def make_tasks():
            t0 = ["D:k:2", "D:v:0", "D:q:2", "D:v:1", "D:k:3",
                  "G:k:2:0", "V:0:0", "V:0:1", "G:q:2:0", "V:0:2",
                  "V:0:3", "D:q:3", "G:k:3:0", "V:1:0", "V:1:1",
                  "V:1:2", "D:v:2", "V:1:3", "G:q:3:0", "V:2:0",
                  "V:2:1", "D:v:3", "V:2:2", "V:2:3", "V:3:0",
                  "V:3:1", "V:3:2", "V:3:3"]
            tasks = {0: t0}
            for i in range(1, 4):
                tasks[i] = [
                    "D:k:0", "D:q:0", "D:k:1",
                    f"G:k:0:{i}", "D:q:1", f"G:q:0:{i}",
                    "D:k:2", f"G:k:1:{i}", "D:k:3", f"G:q:1:{i}",
                    f"G:k:2:{i}", f"G:k:3:{i}",
                ]
                tasks[i + 3] = [
                    "D:q:2", "D:q:3", f"G:q:2:{i}", f"G:q:3:{i}",
                ]
            return tasks

        proj_tasks = make_tasks()
        _src = {"q": qT, "k": kT, "v": vT}
        _dst = {"q": qpT, "k": kpT}

        def run_task(state, task):
            p = task.split(":")
            if p[0] == "D":
                state[(p[1], int(p[2]))] = dma_quarter(_src[p[1]], int(p[2]))
            elif p[0] == "G":
                nm, qu, i = p[1], int(p[2]), int(p[3])
                qk_group(nm, _dst[nm], state[(nm, qu)], qu, i)
            else:
                qu, tt_ = int(p[1]), int(p[2])
                v_group(state[("v", qu)], qu, tt_)

        # ---- pre-step: q & k pd-tile 0, token quarters 0-1 only.
        # Per-queue DMA bandwidth is ~130 GB/s, so the four critical 1MB
        # startup loads are half-split across the three DMA-capable queues
        # (scalar is free until the first exp). Ready order: q e0-3 ~4.5us,
        # q e4-7 / k e0-3 ~9us, k e4-7 ~13us -> first exp ~15us. ----
        xq = inx.tile([P, ET * QW], BF16, tag="inx", name="inx")
        xk = inx.tile([P, ET * QW], BF16, tag="inx", name="inx")
        dma_w_half("q", wq_d, 0, nc.sync)
        dma_w_half("k", wk_d, 0, nc.gpsimd)
        dma_x_half(qT, 0, xq, 0, nc.scalar)
        dma_w_half("q", wq_d, 1, nc.sync)
        dma_x_half(kT, 0, xk, 0, nc.gpsimd)
        dma_x_half(qT, 0, xq, 1, nc.scalar)
        dma_w_half("k", wk_d, 1, nc.sync)
        nc.sync.dma_start(out=bqk["q"][:], in_=bq_d.rearrange("i p -> p i"))
        nc.sync.dma_start(out=bqk["k"][:], in_=bk_d.rearrange("i p -> p i"))
        dma_x_half(kT, 0, xk, 1, nc.gpsimd)
        xq1 = dma_quarter(qT, 1, eng=nc.scalar)
        xk1 = dma_quarter(kT, 1, eng=nc.sync)
        dummy_mms(WARMUP_MMS)
        qk_group("q", qpT, xq, 0, 0)
        qk_group("k", kpT, xk, 0, 0)
        qk_group("q", qpT, xq1, 1, 0)
        qk_group("k", kpT, xk1, 1, 0)
        # hold the non-critical loads (wv/wo + task quarters) until the
        # pre-step casts retire so they don't steal HBM bandwidth from the
        # critical-path startup loads
        nc.multi_engine_barrier(
            [mybir.EngineType.SP, mybir.EngineType.Pool,
             mybir.EngineType.DVE])
        late_loads()

        # ---- output projection ----
        def outproj_tile(t, only_n=None):
            for n in ((0, 1) if only_n is None else (only_n,)):
                ps = ps_pool.tile([P, QW], F32, tag="s", name="ps")
                for c in range(4):
                    nc.tensor.matmul(
                        ps[:], OT[c][:, P * t:P * (t + 1)],
                        wo_sb[:, E * c + QW * n:E * c + QW * (n + 1)],
                        start=(c == 0), stop=(c == 3))
                ost = ost_pool.tile([P, QW], F32, tag="outst", name="outst")
                nc.vector.tensor_copy(ost[:], ps[:])
                eng = nc.sync if n == 0 else nc.gpsimd
                eng.dma_start(
                    out=out[P * t:P * (t + 1), QW * n:QW * (n + 1)],
                    in_=ost[:])

        # ---- attention machinery ----
        def s_granule(cur, saved_cur, gi):
            """S matmuls + exp for granule gi of step cur."""
            hp, blk = cur
            kt, j = gi // 2, gi % 2
            q0 = 1024 * blk + QW * j
            ps = ps_pool.tile([P, 1024], F32, tag="s", name="ps")
            for half in range(2):
                nc.tensor.matmul(
                    ps[:, QW * half:QW * (half + 1)],
                    kpT[hp][64 * half:64 * (half + 1), P * kt:P * (kt + 1)],
                    qpT[hp][64 * half:64 * (half + 1), q0:q0 + QW],
                    start=True, stop=True)
            e = expS.tile([P, 1024], BF16, tag="expS", name="expS")
            nc.scalar.activation(e[:], ps[:], Exp)
            saved_cur[kt][j] = e

        def pv_mms(prev, saved_prev, pv_live, jj, kk, nk=1):
            """PV (M=65: 64 V dims + ones col) for k-tiles kk..kk+nk-1."""
            php, pblk = prev
            if jj not in pv_live:
                pv_live[jj] = [
                    pv_pool.tile([65, QW], F32, tag="pv", name="pv")
                    for _ in range(2)]
            for k2 in range(kk, kk + nk):
                eS = saved_prev[k2][jj]
                for hh in range(2):
                    h = 2 * php + hh
                    nc.tensor.matmul(
                        pv_live[jj][hh][:],
                        vpa[k2][:, 65 * h:65 * (h + 1)],
                        eS[:, QW * hh:QW * (hh + 1)],
                        start=(k2 == 0), stop=(k2 == TT - 1))

        def finish_pv_group(prev, pv_live, jj, den):
            """PV group done: stage unnormalized O^T + denominator rows."""
            php, pblk = prev
            qt = 2 * pblk + jj
            pvh = pv_live.pop(jj)
            for hh in range(2):
                nc.vector.tensor_copy(
                    OT[php][64 * hh:64 * (hh + 1), QW * qt:QW * (qt + 1)],
                    pvh[hh][0:64, :])
                # custom-DVE recip can't route cross-partition reads:
                # stage the PSUM denominator row (partition 64) to an
                # SBUF partition-0 tile now, freeing the PSUM bank.
                st = scs_pool.tile([1, QW], F32, tag="scs", name="scs")
                nc.vector.tensor_copy(st[:], pvh[hh][64:65, :])
                den[2 * jj + hh] = st

        def norm_recip(pend, r):
            # in-place: custom-DVE ops require base partition 0, and the
            # staged [1,512] row is already there.
            st = pend["den"][r]
            nc.vector.reciprocal_approx_fast(st[:], st[:])

        def norm_apply(pend, r):
            jj, hh = r // 2, r % 2
            php, pblk = pend["step"]
            qt = 2 * pblk + jj
            bc = bc_pool.tile([P, QW], F32, tag="bc", name="bc")
            nc.gpsimd.partition_broadcast(bc[:], pend["den"][r][:],
                                          channels=P)
            sl = OT[php][64 * hh:64 * (hh + 1), QW * qt:QW * (qt + 1)]
            nc.vector.tensor_mul(sl, sl, bc[64 * hh:64 * (hh + 1), :])

        # ---- steady-state steps ----
        saved = {}
        prev = None
        norm_pending = None
        for idx, cur in enumerate(STEPS):
            saved[cur] = [[None, None] for _ in range(TT)]
            tasks = proj_tasks.pop(idx, [])
            tstate = {}
            ntask = len(tasks)
            pv_live = {}
            cur_den = {}
            for gi in range(2 * TT):        # 32 granules per step
                # normalization of the step PV'd last phase
                if norm_pending is not None:
                    if 2 <= gi <= 5:
                        norm_recip(norm_pending, gi - 2)
                    elif 6 <= gi <= 9:
                        norm_apply(norm_pending, gi - 6)
                        if gi == 9:
                            norm_pending = None
                # PV of prev step (emitted before S so a stalled S
                # matmul never blocks ready PV work in the engine queue).
                if prev is not None:
                    jj, kk = gi // TT, gi % TT
                    pv_mms(prev, saved[prev], pv_live, jj, kk)
                    if kk == TT - 1:
                        finish_pv_group(prev, pv_live, jj, cur_den)
                # S + exp of current step
                s_granule(cur, saved[cur], gi)
                # interleaved projection tasks
                if ntask:
                    t0 = ntask * gi // (2 * TT)
                    t1 = ntask * (gi + 1) // (2 * TT)
                    for ti in range(t0, t1):
                        run_task(tstate, tasks[ti])
                # output projection of q-block 0 rides the PV-free second
                # halves of steps 6-7 (granules 16+ carry only S + norm
                # work), as 4-matmul half-bursts so S never waits long
                if idx == 6 and 16 <= gi <= 25:
                    s = gi - 16
                    outproj_tile(s // 2, only_n=s % 2)
                elif idx == 7 and 16 <= gi <= 21:
                    s = gi - 16
                    outproj_tile(5 + s // 2, only_n=s % 2)
            if prev is not None:
                norm_pending = {"step": prev, "den": cur_den}
            prev = cur
            if idx >= 2:
                saved.pop(STEPS[idx - 2], None)

        # ---- drain: PV + norm of the last step interleaved with the
        # remaining output projection (jj0 denominators stage at granule
        # 7, so its norms and the qt2 outproj tiles ride granules 8-14).
        pv_live = {}
        pend31 = {"step": prev, "den": {}}
        wdrain = ps_pool.tile([P, QW], F32, tag="s", name="wdrain")
        for gi in range(TT):                # compressed: 2 k-tiles/granule
            if norm_pending is not None:
                if 0 <= gi <= 3:
                    norm_recip(norm_pending, gi)
                elif 4 <= gi <= 7:
                    norm_apply(norm_pending, gi - 4)
                    if gi == 7:
                        norm_pending = None
            jj, kk = gi // 8, 2 * (gi % 8)
            pv_mms(prev, saved[prev], pv_live, jj, kk, nk=2)
            if kk == TT - 2:
                finish_pv_group(prev, pv_live, jj, pend31["den"])
            if gi == 8:
                norm_recip(pend31, 0)
                norm_recip(pend31, 1)
                dummy_mms(6, tile=wdrain)
            elif gi == 9:
                norm_apply(pend31, 0)
                norm_apply(pend31, 1)
                dummy_mms(6, tile=wdrain)
            elif gi == 10:
                dummy_mms(6, tile=wdrain)
            elif 11 <= gi <= 14:
                outproj_tile(8 + (gi - 11))
        # jj1 denominators stage at granule 15 (PV stop); norm + project
        # the last quarter once they land.
        norm_recip(pend31, 2)
        norm_recip(pend31, 3)
        norm_apply(pend31, 2)
        norm_apply(pend31, 3)
        for t in range(12, 16):
            outproj_tile(t)

def build_nc():
    nc = bacc.Bacc("TRN2", target_bir_lowering=False, debug=False,
                   num_devices=NCORES)
    io = {
        "qT": nc.dram_tensor("qT", [E, L], BF16, kind="ExternalInput").ap(),
        "kT": nc.dram_tensor("kT", [E, L], BF16, kind="ExternalInput").ap(),
        "vT": nc.dram_tensor("vT", [E, L], BF16, kind="ExternalInput").ap(),
        "wq": nc.dram_tensor("wq", [E, D], BF16, kind="ExternalInput").ap(),
        "wk": nc.dram_tensor("wk", [E, D], BF16, kind="ExternalInput").ap(),
        "wv": nc.dram_tensor("wv", [E, D], BF16, kind="ExternalInput").ap(),
        "wo": nc.dram_tensor("wo", [D, E], BF16, kind="ExternalInput").ap(),
        "bq": nc.dram_tensor("bq", [PT, P], F32, kind="ExternalInput").ap(),
        "bk": nc.dram_tensor("bk", [PT, P], F32, kind="ExternalInput").ap(),
        "bv": nc.dram_tensor("bv", [1, D], F32, kind="ExternalInput").ap(),
        "out": nc.dram_tensor("out", [L, E], F32,
                              kind="ExternalOutput").ap(),
    }
    with tile.TileContext(nc) as tc:
        _emit(nc, tc, io)
    nc.compile()
    return nc


_NC = None


def _get_nc():
    global _NC
    if _NC is None:
        _NC = build_nc()
    return _NC


def make_in_maps(q, k, v, Wq, bq, Wk, bk, Wv, bv, Wo):
    scale = np.float32(1.0 / math.sqrt(HD))
    in_maps = []
    for c in range(NCORES):
        b, g = divmod(c, 2)
        sl = slice(g * D, (g + 1) * D)
        in_maps.append({
            "qT": np.ascontiguousarray(q[b].T).astype(BF16_NP),
            "kT": np.ascontiguousarray(k[b].T).astype(BF16_NP),
            "vT": np.ascontiguousarray(v[b].T).astype(BF16_NP),
            "wq": (Wq[:, sl] * scale).astype(BF16_NP),
            "wk": np.ascontiguousarray(Wk[:, sl]).astype(BF16_NP),
            "wv": np.ascontiguousarray(Wv[:, sl]).astype(BF16_NP),
            "wo": np.ascontiguousarray(Wo[sl, :]).astype(BF16_NP),
            "bq": (bq[sl] * scale).reshape(PT, P).astype(np.float32),
            "bk": bk[sl].reshape(PT, P).astype(np.float32),
            "bv": bv[sl].reshape(1, D).astype(np.float32),
        })
    return in_maps


def kernel(q, k, v, mask, Wq, bq, Wk, bk, Wv, bv, Wo, bo):
    global LAST_EXEC_NS, LAST_RESULTS
    q, k, v = (np.asarray(x, np.float32) for x in (q, k, v))
    Wq, bq, Wk, bk, Wv, bv, Wo, bo = (
        np.asarray(x, np.float32)
        for x in (Wq, bq, Wk, bk, Wv, bv, Wo, bo))
    nc = _get_nc()
    in_maps = make_in_maps(q, k, v, Wq, bq, Wk, bk, Wv, bv, Wo)
    kwargs = {}
    if TRACE:
        kwargs = dict(trace=True)
    res = run_bass_kernel_spmd(nc, in_maps, list(range(NCORES)), **kwargs)
    LAST_EXEC_NS = res.exec_time_ns
    LAST_RESULTS = res
    outs = [np.asarray(res.results[c]["out"], np.float32)
            for c in range(NCORES)]
    full = np.stack([outs[2 * b] + outs[2 * b + 1] for b in range(B)], axis=0)
    full += bo[None, None, :].astype(np.float32)
    return full.astype(np.float32)



# revision 26
# speedup vs baseline: 1.0908x; 1.0908x over previous
"""MultiHeadedAttention Trainium2 kernel (8 NeuronCores).

Sharding: core c -> (batch b = c//2, head-group g = c%2). Each core computes
the 8-head attention slice for one batch plus its partial output projection;
the host sums the two partials per batch and adds the output bias.

Device-side layout is feature-major: the host ships q/k/v pre-transposed
([E, L], bf16) so every matmul contracts along SBUF partitions without any
on-chip transpose. The attention scale 1/sqrt(hd) is folded into Wq/bq on
the host. Projection biases are applied during the PSUM->SBUF cast via DVE
tensor_scalar (per-partition bias AP for q/k; broadcast row for v).

The kernel is paced by PE matmul columns and the ScalarE exp of the 256
score tiles ([128,1024] each, ~1.15us apiece), which are roughly
balanced; everything else is scheduled to hide under that wall:
  - dummy warmup matmuls + single-descriptor [128,4096] DMAs keep the PE
    HAM clock-gate warm from t~1us (2.4GHz instead of 1.2).
  - softmax denominators ride the PV matmuls for free: each head's V
    slice carries a ones column (per-head width 65), so the M=65 PV
    accumulation leaves sum_k exp(S) in PSUM row 64. This removes the
    512 M=1 ones-matmuls (~109us of PE time) the old design spent on
    denominators. Normalization: reciprocal_approx_fast + gpsimd
    partition_broadcast + one DVE multiply per (head, q-block).
  - q/k/v projections interleave into the early steps' PE slack;
    the output projection of q-block 0 overlaps the last step.
"""

import math
import sys

sys.path.insert(0, "/opt/trn_rl_repo")

import numpy as np
import ml_dtypes

import concourse.bass as bass  # noqa: F401  (registers rust bindings)
import concourse.mybir as mybir
import concourse.tile as tile
from concourse import bacc
from concourse.bass_utils import run_bass_kernel_spmd

BF16_NP = ml_dtypes.bfloat16
F32 = mybir.dt.float32
BF16 = mybir.dt.bfloat16

B, L, E, H, HD = 4, 2048, 1024, 16, 64
NCORES = 8
D = 512          # per-core projection width (8 heads * 64)
P = 128
ET = E // P      # 8 contraction tiles over E
PT = D // P      # 4 partition-tiles of qpT/kpT
TT = L // P      # 16 token tiles
QW = 512         # matmul moving free width

WARMUP_MMS = 40
EXPS_BUFS = 33

TRACE = False
LAST_EXEC_NS = None
LAST_RESULTS = None

# step order: (head-pair, q-block). Pair 3 blk 0 runs at idx 4 so its PV
# (idx 5) and norm (idx 6 g2-9) finish early enough for the blk-0 output
# projection to ride the idle PE slack of steps 6-7.
STEPS = [(0, 0), (0, 1), (1, 0), (2, 0), (3, 0), (1, 1), (2, 1), (3, 1)]

Add = mybir.AluOpType.add


def _emit(nc, tc, io):
    Exp = mybir.ActivationFunctionType.Exp
    qT, kT, vT = io["qT"], io["kT"], io["vT"]
    wq_d, wk_d, wv_d, wo_d = io["wq"], io["wk"], io["wv"], io["wo"]
    bq_d, bk_d = io["bq"], io["bk"]
    out = io["out"]

    import contextlib
    stack = contextlib.ExitStack()
    with stack:
        pers = stack.enter_context(tc.tile_pool(name="pers", bufs=1))
        inx = stack.enter_context(tc.tile_pool(name="inx", bufs=3))
        expS = stack.enter_context(tc.tile_pool(name="expS", bufs=EXPS_BUFS))
        ps_pool = stack.enter_context(
            tc.tile_pool(name="ps", bufs=2, space="PSUM"))
        pv_pool = stack.enter_context(
            tc.tile_pool(name="pv", bufs=4, space="PSUM"))
        ost_pool = stack.enter_context(tc.tile_pool(name="ost", bufs=2))
        scs_pool = stack.enter_context(tc.tile_pool(name="scs", bufs=4))
        bc_pool = stack.enter_context(tc.tile_pool(name="bc", bufs=1))

        # ---- persistent SBUF ----
        qpT = [pers.tile([P, L], BF16, tag=f"qpT{i}", name=f"qpT{i}")
               for i in range(PT)]
        kpT = [pers.tile([P, L], BF16, tag=f"kpT{i}", name=f"kpT{i}")
               for i in range(PT)]
        OT = [pers.tile([P, L], BF16, tag=f"OT{i}", name=f"OT{i}")
              for i in range(PT)]
        # per-head width 65: cols 65h..65h+63 = V dims, col 65h+64 = 1.0
        # (the ones column makes the M=65 PV matmul accumulate the softmax
        # denominator in PSUM row 64)
        vpa = [pers.tile([P, H // 2 * 65], BF16, tag=f"vpa{t}", name=f"vpa{t}")
               for t in range(TT)]
        w_sb = {nm: pers.tile([P, ET * D], BF16, tag=f"w{nm}", name=f"w{nm}")
                for nm in ("q", "k", "v")}
        wo_sb = pers.tile([P, 4 * E], BF16, tag="wo", name="wo")
        bqk = {nm: pers.tile([P, PT], F32, tag=f"b{nm}", name=f"b{nm}")
               for nm in ("q", "k")}
        scratch = pers.tile([P, 256], BF16, tag="scr", name="scr")

        # ---- warmup: keep the PE busy (and the HAM clock-gate warming)
        # while the first weight/input DMAs land ----
        nc.vector.memset(scratch[:], 0.125)
        for t in range(TT):
            nc.vector.memset(
                vpa[t].rearrange("p (h c) -> p h c", c=65)[:, :, 64:65], 1.0)
        def dummy_mms(n, tile=None):
            wt = tile if tile is not None else ps_pool.tile(
                [P, QW], F32, tag="s", name="warm")
            for _ in range(n):
                nc.tensor.matmul(wt[:, 0:256], scratch[:, 0:P],
                                 scratch[:], start=True, stop=True)

        # ---- weight loads ----
        # weights ship host-pre-tiled as [p][e][d] (and wo as [p][c][e]),
        # matching the SBUF free layout exactly: every transfer is one
        # fully-contiguous descriptor with 4-8KB partition lines instead
        # of 1024 separate 1KB lines.
        def dma_w_half(nm, wdram, h, eng):
            eng.dma_start(
                out=w_sb[nm][:, 4 * D * h:4 * D * (h + 1)],
                in_=wdram[:, 4 * D * h:4 * D * (h + 1)])

        def dma_x_half(xd, qu, xt, h, eng):
            eng.dma_start(
                out=xt[:, 4 * QW * h:4 * QW * (h + 1)],
                in_=xd[qu][:, 4 * QW * h:4 * QW * (h + 1)])

        def late_loads():
            nc.gpsimd.dma_start(out=w_sb["v"][:], in_=wv_d)
            nc.gpsimd.dma_start(out=wo_sb[:], in_=wo_d)

        # ---- projection building blocks ----
        # Per-queue DMA bandwidth is only ~130 GB/s, so spread the 1MB
        # quarter loads across three issue queues (scalar is reserved: an
        # exp stalled behind a DMA issue costs wall time directly).
        dma_flip = [0]
        _qrot = (nc.sync, nc.gpsimd)

        def dma_quarter(xd, qu, eng=None):
            xt = inx.tile([P, ET * QW], BF16, tag="inx", name="inx")
            if eng is None:
                eng = _qrot[dma_flip[0] % 2]
                dma_flip[0] += 1
            eng.dma_start(out=xt[:], in_=xd[qu])
            return xt

        def qk_group(nm, dst, xt, qu, i):
            """One psum group: qpT/kpT pd-tile i, token quarter qu.

            PSUM comes from the shared 1-bank "pv" tag so the 2-slot S
            rotation is never blocked behind a projection group."""
            ps = pv_pool.tile([P, QW], F32, tag="pv", name="ps")
            for e in range(ET):
                nc.tensor.matmul(
                    ps[:], w_sb[nm][:, D * e + P * i:D * e + P * (i + 1)],
                    xt[:, QW * e:QW * (e + 1)],
                    start=(e == 0), stop=(e == ET - 1))
            nc.vector.tensor_scalar(
                dst[i][:, QW * qu:QW * (qu + 1)], ps[:],
                bqk[nm][:, i:i + 1], None, Add)

        def v_group(xt, qu, tt_):
            t = 4 * qu + tt_
            ps = pv_pool.tile([P, D], F32, tag="pv", name="ps")
            for e in range(ET):
                nc.tensor.matmul(
                    ps[:], xt[:, QW * e + P * tt_:QW * e + P * (tt_ + 1)],
                    w_sb["v"][:, D * e:D * (e + 1)],
                    start=(e == 0), stop=(e == ET - 1))
            # bv is folded into bo on the host (softmax weights sum to 1,
            # so +bv per value adds exactly bv @ Wo to every token)
            nc.vector.tensor_copy(
                vpa[t].rearrange("p (h c) -> p h c", c=65)[:, :, 0:64],
                ps[:].rearrange("p (h c) -> p h c", c=64))

        # Interleaved projection task lists, one per early attention step.
        # Tokens: "D:nm:qu" = quarter DMA, "G:nm:qu:i" = qk group,
        # "V:qu:tt" = v group. Quarter DMAs sit ~6 task slots ahead of
        # their consumer group (a 1MB transfer takes ~7us on contended HBM).
        def make_tasks():
            t0 = ["D:k:2", "D:v:0", "D:q:2", "D:v:1", "D:k:3",
                  "G:k:2:0", "V:0:0", "V:0:1", "G:q:2:0", "V:0:2",
                  "V:0:3", "D:q:3", "G:k:3:0", "V:1:0", "V:1:1",
                  "V:1:2", "D:v:2", "V:1:3", "G:q:3:0", "V:2:0",
                  "V:2:1", "D:v:3", "V:2:2", "V:2:3", "V:3:0",
                  "V:3:1", "V:3:2", "V:3:3"]
            tasks = {0: t0}
            for i in range(1, 4):
                tasks[i] = [
                    "D:k:0", "D:q:0", "D:k:1",
                    f"G:k:0:{i}", "D:q:1", f"G:q:0:{i}",
                    "D:k:2", f"G:k:1:{i}", "D:k:3", f"G:q:1:{i}",
                    f"G:k:2:{i}", f"G:k:3:{i}",
                ]
                tasks[i + 3] = [
                    "D:q:2", "D:q:3", f"G:q:2:{i}", f"G:q:3:{i}",
                ]
            return tasks

        proj_tasks = make_tasks()
        _src = {"q": qT, "k": kT, "v": vT}
        _dst = {"q": qpT, "k": kpT}

        def run_task(state, task):
            p = task.split(":")
            if p[0] == "D":
                state[(p[1], int(p[2]))] = dma_quarter(_src[p[1]], int(p[2]))
            elif p[0] == "G":
                nm, qu, i = p[1], int(p[2]), int(p[3])
                qk_group(nm, _dst[nm], state[(nm, qu)], qu, i)
            else:
                qu, tt_ = int(p[1]), int(p[2])
                v_group(state[("v", qu)], qu, tt_)

        # ---- pre-step: q & k pd-tile 0, token quarters 0-1 only.
        # Per-queue DMA bandwidth is ~130 GB/s, so the four critical 1MB
        # startup loads are half-split across the three DMA-capable queues
        # (scalar is free until the first exp). Ready order: q e0-3 ~4.5us,
        # q e4-7 / k e0-3 ~9us, k e4-7 ~13us -> first exp ~15us. ----
        xq = inx.tile([P, ET * QW], BF16, tag="inx", name="inx")
        xk = inx.tile([P, ET * QW], BF16, tag="inx", name="inx")
        dma_w_half("q", wq_d, 0, nc.sync)
        dma_w_half("k", wk_d, 0, nc.gpsimd)
        dma_x_half(qT, 0, xq, 0, nc.scalar)
        dma_w_half("q", wq_d, 1, nc.sync)
        dma_x_half(kT, 0, xk, 0, nc.gpsimd)
        dma_x_half(qT, 0, xq, 1, nc.scalar)
        dma_w_half("k", wk_d, 1, nc.sync)
        nc.sync.dma_start(out=bqk["q"][:], in_=bq_d.rearrange("i p -> p i"))
        nc.sync.dma_start(out=bqk["k"][:], in_=bk_d.rearrange("i p -> p i"))
        dma_x_half(kT, 0, xk, 1, nc.gpsimd)
        xq1 = dma_quarter(qT, 1, eng=nc.scalar)
        xk1 = dma_quarter(kT, 1, eng=nc.sync)
        dummy_mms(WARMUP_MMS)
        qk_group("q", qpT, xq, 0, 0)
        qk_group("k", kpT, xk, 0, 0)
        qk_group("q", qpT, xq1, 1, 0)
        qk_group("k", kpT, xk1, 1, 0)
        # hold the non-critical loads (wv/wo + task quarters) until the
        # pre-step casts retire so they don't steal HBM bandwidth from the
        # critical-path startup loads
        nc.multi_engine_barrier(
            [mybir.EngineType.SP, mybir.EngineType.Pool,
             mybir.EngineType.DVE])
        late_loads()

        # ---- output projection ----
        def outproj_tile(t, only_n=None):
            for n in ((0, 1) if only_n is None else (only_n,)):
                ps = pv_pool.tile([P, QW], F32, tag="pv", name="ps")
                for c in range(4):
                    nc.tensor.matmul(
                        ps[:], OT[c][:, P * t:P * (t + 1)],
                        wo_sb[:, E * c + QW * n:E * c + QW * (n + 1)],
                        start=(c == 0), stop=(c == 3))
                ost = ost_pool.tile([P, QW], F32, tag="outst", name="outst")
                nc.vector.tensor_copy(ost[:], ps[:])
                eng = nc.sync if n == 0 else nc.gpsimd
                eng.dma_start(
                    out=out[P * t:P * (t + 1), QW * n:QW * (n + 1)],
                    in_=ost[:])

        # ---- attention machinery ----
        def s_granule(cur, saved_cur, gi):
            """S matmuls + exp for granule gi of step cur."""
            hp, blk = cur
            kt, j = gi // 2, gi % 2
            q0 = 1024 * blk + QW * j
            ps = ps_pool.tile([P, 1024], F32, tag="s", name="ps")
            for half in range(2):
                nc.tensor.matmul(
                    ps[:, QW * half:QW * (half + 1)],
                    kpT[hp][64 * half:64 * (half + 1), P * kt:P * (kt + 1)],
                    qpT[hp][64 * half:64 * (half + 1), q0:q0 + QW],
                    start=True, stop=True)
            e = expS.tile([P, 1024], BF16, tag="expS", name="expS")
            nc.scalar.activation(e[:], ps[:], Exp)
            saved_cur[kt][j] = e

        def pv_mms(prev, saved_prev, pv_live, jj, kk, nk=1):
            """PV (M=65: 64 V dims + ones col) for k-tiles kk..kk+nk-1."""
            php, pblk = prev
            if jj not in pv_live:
                pv_live[jj] = [
                    pv_pool.tile([65, QW], F32, tag="pv", name="pv")
                    for _ in range(2)]
            for k2 in range(kk, kk + nk):
                eS = saved_prev[k2][jj]
                for hh in range(2):
                    h = 2 * php + hh
                    nc.tensor.matmul(
                        pv_live[jj][hh][:],
                        vpa[k2][:, 65 * h:65 * (h + 1)],
                        eS[:, QW * hh:QW * (hh + 1)],
                        start=(k2 == 0), stop=(k2 == TT - 1))

        def finish_pv_group(prev, pv_live, jj, den):
            """PV group done: stage unnormalized O^T + denominator rows."""
            php, pblk = prev
            qt = 2 * pblk + jj
            pvh = pv_live.pop(jj)
            for hh in range(2):
                nc.vector.tensor_copy(
                    OT[php][64 * hh:64 * (hh + 1), QW * qt:QW * (qt + 1)],
                    pvh[hh][0:64, :])
                # custom-DVE recip can't route cross-partition reads:
                # stage the PSUM denominator row (partition 64) to an
                # SBUF partition-0 tile now, freeing the PSUM bank.
                st = scs_pool.tile([1, QW], F32, tag="scs", name="scs")
                nc.vector.tensor_copy(st[:], pvh[hh][64:65, :])
                den[2 * jj + hh] = st

        def norm_recip(pend, r):
            # in-place: custom-DVE ops require base partition 0, and the
            # staged [1,512] row is already there.
            st = pend["den"][r]
            nc.vector.reciprocal_approx_fast(st[:], st[:])

        def norm_apply(pend, r):
            jj, hh = r // 2, r % 2
            php, pblk = pend["step"]
            qt = 2 * pblk + jj
            bc = bc_pool.tile([P, QW], F32, tag="bc", name="bc")
            nc.gpsimd.partition_broadcast(bc[:], pend["den"][r][:],
                                          channels=P)
            sl = OT[php][64 * hh:64 * (hh + 1), QW * qt:QW * (qt + 1)]
            nc.vector.tensor_mul(sl, sl, bc[64 * hh:64 * (hh + 1), :])

        # ---- steady-state steps ----
        saved = {}
        prev = None
        norm_pending = None
        for idx, cur in enumerate(STEPS):
            saved[cur] = [[None, None] for _ in range(TT)]
            tasks = proj_tasks.pop(idx, [])
            tstate = {}
            ntask = len(tasks)
            pv_live = {}
            cur_den = {}
            for gi in range(2 * TT):        # 32 granules per step
                # normalization of the step PV'd last phase
                if norm_pending is not None:
                    if 2 <= gi <= 5:
                        norm_recip(norm_pending, gi - 2)
                    elif 6 <= gi <= 9:
                        norm_apply(norm_pending, gi - 6)
                        if gi == 9:
                            norm_pending = None
                # PV of prev step (emitted before S so a stalled S
                # matmul never blocks ready PV work in the engine queue).
                if prev is not None:
                    jj, kk = gi // TT, gi % TT
                    pv_mms(prev, saved[prev], pv_live, jj, kk)
                    if kk == TT - 1:
                        finish_pv_group(prev, pv_live, jj, cur_den)
                # S + exp of current step
                s_granule(cur, saved[cur], gi)
                # interleaved projection tasks (step 6 packs them into the
                # first half so they never overlap the outproj "pv"-tag
                # allocations of granules 16+: at most 4 pv slots live)
                if ntask:
                    tw = TT if idx == 6 else 2 * TT
                    if gi < tw:
                        t0 = ntask * gi // tw
                        t1 = ntask * (gi + 1) // tw
                        for ti in range(t0, t1):
                            run_task(tstate, tasks[ti])
                # output projection of q-block 0 rides the PV-free second
                # halves of steps 6-7 (granules 16+ carry only S + norm
                # work), as 4-matmul half-bursts so S never waits long
                if idx == 6 and 16 <= gi <= 25:
                    s = gi - 16
                    outproj_tile(s // 2, only_n=s % 2)
                elif idx == 7 and 16 <= gi <= 21:
                    s = gi - 16
                    outproj_tile(5 + s // 2, only_n=s % 2)
            if prev is not None:
                norm_pending = {"step": prev, "den": cur_den}
            prev = cur
            if idx >= 2:
                saved.pop(STEPS[idx - 2], None)

        # ---- drain: PV + norm of the last step interleaved with the
        # remaining output projection (jj0 denominators stage at granule
        # 7, so its norms and the qt2 outproj tiles ride granules 8-14).
        pv_live = {}
        pend31 = {"step": prev, "den": {}}
        wdrain = ps_pool.tile([P, QW], F32, tag="s", name="wdrain")
        for gi in range(TT):                # compressed: 2 k-tiles/granule
            if norm_pending is not None:
                if 0 <= gi <= 3:
                    norm_recip(norm_pending, gi)
                elif 4 <= gi <= 7:
                    norm_apply(norm_pending, gi - 4)
                    if gi == 7:
                        norm_pending = None
            jj, kk = gi // 8, 2 * (gi % 8)
            pv_mms(prev, saved[prev], pv_live, jj, kk, nk=2)
            if kk == TT - 2:
                finish_pv_group(prev, pv_live, jj, pend31["den"])
            if gi == 8:
                norm_recip(pend31, 0)
                norm_recip(pend31, 1)
                dummy_mms(6, tile=wdrain)
            elif gi == 9:
                norm_apply(pend31, 0)
                norm_apply(pend31, 1)
                dummy_mms(6, tile=wdrain)
            elif gi == 10:
                dummy_mms(6, tile=wdrain)
            elif 11 <= gi <= 14:
                outproj_tile(8 + (gi - 11))
        # jj1 denominators stage at granule 15 (PV stop); norm + project
        # the last quarter once they land.
        norm_recip(pend31, 2)
        norm_recip(pend31, 3)
        norm_apply(pend31, 2)
        norm_apply(pend31, 3)
        for t in range(12, 16):
            outproj_tile(t)

def build_nc():
    nc = bacc.Bacc("TRN2", target_bir_lowering=False, debug=False,
                   num_devices=NCORES)
    # q/k/v ship as [quarter][partition][e-chunk*512t] and weights as
    # [partition][e-chunk*512d] ([p][c*1024e] for wo): SBUF-layout-exact,
    # so every load is one contiguous descriptor with 4-8KB lines.
    io = {
        "qT": nc.dram_tensor("qT", [4, P, ET * QW], BF16,
                             kind="ExternalInput").ap(),
        "kT": nc.dram_tensor("kT", [4, P, ET * QW], BF16,
                             kind="ExternalInput").ap(),
        "vT": nc.dram_tensor("vT", [4, P, ET * QW], BF16,
                             kind="ExternalInput").ap(),
        "wq": nc.dram_tensor("wq", [P, ET * D], BF16,
                             kind="ExternalInput").ap(),
        "wk": nc.dram_tensor("wk", [P, ET * D], BF16,
                             kind="ExternalInput").ap(),
        "wv": nc.dram_tensor("wv", [P, ET * D], BF16,
                             kind="ExternalInput").ap(),
        "wo": nc.dram_tensor("wo", [P, 4 * E], BF16,
                             kind="ExternalInput").ap(),
        "bq": nc.dram_tensor("bq", [PT, P], F32, kind="ExternalInput").ap(),
        "bk": nc.dram_tensor("bk", [PT, P], F32, kind="ExternalInput").ap(),
        "out": nc.dram_tensor("out", [L, E], F32,
                              kind="ExternalOutput").ap(),
    }
    with tile.TileContext(nc) as tc:
        _emit(nc, tc, io)
    nc.compile()
    return nc


_NC = None


def _get_nc():
    global _NC
    if _NC is None:
        _NC = build_nc()
    return _NC


def _x4(xb):
    """[L, E] batch slice -> [quarter, p, e*512t] device layout (bf16)."""
    # element (qu, p, e, t) = x.T[128e + p, 512qu + t]
    a = xb.T.reshape(ET, P, 4, QW).transpose(2, 1, 0, 3)
    return np.ascontiguousarray(a.reshape(4, P, ET * QW)).astype(BF16_NP)


def _w4(w):
    """[E, D] weight slice -> [p, e*512d] device layout (bf16)."""
    a = w.reshape(ET, P, D).transpose(1, 0, 2)
    return np.ascontiguousarray(a.reshape(P, ET * D)).astype(BF16_NP)


def make_in_maps(q, k, v, Wq, bq, Wk, bk, Wv, Wo):
    scale = np.float32(1.0 / math.sqrt(HD))
    in_maps = []
    for c in range(NCORES):
        b, g = divmod(c, 2)
        sl = slice(g * D, (g + 1) * D)
        wo4 = Wo[sl, :].reshape(4, P, E).transpose(1, 0, 2)
        in_maps.append({
            "qT": _x4(q[b]),
            "kT": _x4(k[b]),
            "vT": _x4(v[b]),
            "wq": _w4(Wq[:, sl] * scale),
            "wk": _w4(Wk[:, sl]),
            "wv": _w4(Wv[:, sl]),
            "wo": np.ascontiguousarray(
                wo4.reshape(P, 4 * E)).astype(BF16_NP),
            "bq": (bq[sl] * scale).reshape(PT, P).astype(np.float32),
            "bk": bk[sl].reshape(PT, P).astype(np.float32),
        })
    return in_maps


def kernel(q, k, v, mask, Wq, bq, Wk, bk, Wv, bv, Wo, bo):
    global LAST_EXEC_NS, LAST_RESULTS
    q, k, v = (np.asarray(x, np.float32) for x in (q, k, v))
    Wq, bq, Wk, bk, Wv, bv, Wo, bo = (
        np.asarray(x, np.float32)
        for x in (Wq, bq, Wk, bk, Wv, bv, Wo, bo))
    nc = _get_nc()
    in_maps = make_in_maps(q, k, v, Wq, bq, Wk, bk, Wv, Wo)
    kwargs = {}
    if TRACE:
        kwargs = dict(trace=True)
    res = run_bass_kernel_spmd(nc, in_maps, list(range(NCORES)), **kwargs)
    LAST_EXEC_NS = res.exec_time_ns
    LAST_RESULTS = res
    outs = [np.asarray(res.results[c]["out"], np.float32)
            for c in range(NCORES)]
    full = np.stack([outs[2 * b] + outs[2 * b + 1] for b in range(B)], axis=0)
    # bv folded: softmax rows sum to 1, so +bv on v adds bv @ Wo to
    # every token
    full += (bo + bv @ Wo)[None, None, :].astype(np.float32)
    return full.astype(np.float32)



# revision 31
# speedup vs baseline: 1.1122x; 1.0196x over previous
"""MultiHeadedAttention Trainium2 kernel (8 NeuronCores).

Sharding: core c -> (batch b = c//2, head-group g = c%2). Each core computes
the 8-head attention slice for one batch plus its partial output projection;
the host sums the two partials per batch and adds the output bias.

Device-side layout is feature-major: the host ships q/k/v pre-transposed
([E, L], bf16) so every matmul contracts along SBUF partitions without any
on-chip transpose. The attention scale 1/sqrt(hd) is folded into Wq/bq on
the host. Projection biases are applied during the PSUM->SBUF cast via DVE
tensor_scalar (per-partition bias AP for q/k; broadcast row for v).

The kernel is paced by PE matmul columns and the ScalarE exp of the 256
score tiles ([128,1024] each, ~1.15us apiece), which are roughly
balanced; everything else is scheduled to hide under that wall:
  - dummy warmup matmuls + single-descriptor [128,4096] DMAs keep the PE
    HAM clock-gate warm from t~1us (2.4GHz instead of 1.2).
  - softmax denominators ride the PV matmuls for free: each head's V
    slice carries a ones column (per-head width 65), so the M=65 PV
    accumulation leaves sum_k exp(S) in PSUM row 64. This removes the
    512 M=1 ones-matmuls (~109us of PE time) the old design spent on
    denominators. Normalization: reciprocal_approx_fast + gpsimd
    partition_broadcast + one DVE multiply per (head, q-block).
  - q/k/v projections interleave into the early steps' PE slack;
    the output projection of q-block 0 overlaps the last step.
"""

import math
import sys

sys.path.insert(0, "/opt/trn_rl_repo")

import numpy as np
import ml_dtypes

import concourse.bass as bass  # noqa: F401  (registers rust bindings)
import concourse.mybir as mybir
import concourse.tile as tile
from concourse import bacc
from concourse.bass_utils import run_bass_kernel_spmd

BF16_NP = ml_dtypes.bfloat16
F32 = mybir.dt.float32
BF16 = mybir.dt.bfloat16

B, L, E, H, HD = 4, 2048, 1024, 16, 64
NCORES = 8
D = 512          # per-core projection width (8 heads * 64)
P = 128
ET = E // P      # 8 contraction tiles over E
PT = D // P      # 4 partition-tiles of qpT/kpT
TT = L // P      # 16 token tiles
QW = 512         # matmul moving free width

WARMUP_MMS = 40
EXPS_BUFS = 20
LAG = 16      # PV trails S by this many granules

TRACE = False
LAST_EXEC_NS = None
LAST_RESULTS = None

# step order: (head-pair, q-block). Pair 3 blk 0 runs at idx 4 so its PV
# (idx 5) and norm (idx 6 g2-9) finish early enough for the blk-0 output
# projection to ride the idle PE slack of steps 6-7.
STEPS = [(0, 0), (0, 1), (1, 0), (2, 0), (3, 0), (1, 1), (2, 1), (3, 1)]

Add = mybir.AluOpType.add


def _emit(nc, tc, io):
    Exp = mybir.ActivationFunctionType.Exp
    qT, kT, vT = io["qT"], io["kT"], io["vT"]
    wq_d, wk_d, wv_d, wo_d = io["wq"], io["wk"], io["wv"], io["wo"]
    bq_d, bk_d = io["bq"], io["bk"]
    out = io["out"]

    import contextlib
    stack = contextlib.ExitStack()
    with stack:
        pers = stack.enter_context(tc.tile_pool(name="pers", bufs=1))
        inx = stack.enter_context(tc.tile_pool(name="inx", bufs=3))
        expS = stack.enter_context(tc.tile_pool(name="expS", bufs=EXPS_BUFS))
        ps_pool = stack.enter_context(
            tc.tile_pool(name="ps", bufs=2, space="PSUM"))
        pv_pool = stack.enter_context(
            tc.tile_pool(name="pv", bufs=4, space="PSUM"))
        ost_pool = stack.enter_context(tc.tile_pool(name="ost", bufs=2))
        scs_pool = stack.enter_context(tc.tile_pool(name="scs", bufs=4))
        bc_pool = stack.enter_context(tc.tile_pool(name="bc", bufs=1))

        # ---- persistent SBUF ----
        qpT = [pers.tile([P, L], BF16, tag=f"qpT{i}", name=f"qpT{i}")
               for i in range(PT)]
        kpT = [pers.tile([P, L], BF16, tag=f"kpT{i}", name=f"kpT{i}")
               for i in range(PT)]
        OT = [pers.tile([P, L], BF16, tag=f"OT{i}", name=f"OT{i}")
              for i in range(PT)]
        # per-head width 65: cols 65h..65h+63 = V dims, col 65h+64 = 1.0
        # (the ones column makes the M=65 PV matmul accumulate the softmax
        # denominator in PSUM row 64)
        vpa = [pers.tile([P, H // 2 * 65], BF16, tag=f"vpa{t}", name=f"vpa{t}")
               for t in range(TT)]
        w_sb = {nm: pers.tile([P, ET * D], BF16, tag=f"w{nm}", name=f"w{nm}")
                for nm in ("q", "k", "v")}
        wo_sb = pers.tile([P, 4 * E], BF16, tag="wo", name="wo")
        bqk = {nm: pers.tile([P, PT], F32, tag=f"b{nm}", name=f"b{nm}")
               for nm in ("q", "k")}
        scratch = pers.tile([P, 256], BF16, tag="scr", name="scr")

        # ---- warmup: keep the PE busy (and the HAM clock-gate warming)
        # while the first weight/input DMAs land ----
        nc.vector.memset(scratch[:], 0.125)
        for t in range(TT):
            nc.vector.memset(
                vpa[t].rearrange("p (h c) -> p h c", c=65)[:, :, 64:65], 1.0)
        def dummy_mms(n, tile=None):
            wt = tile if tile is not None else ps_pool.tile(
                [P, QW], F32, tag="s", name="warm")
            for _ in range(n):
                nc.tensor.matmul(wt[:, 0:256], scratch[:, 0:P],
                                 scratch[:], start=True, stop=True)

        # ---- weight loads ----
        # weights ship host-pre-tiled as [p][e][d] (and wo as [p][c][e]),
        # matching the SBUF free layout exactly: every transfer is one
        # fully-contiguous descriptor with 4-8KB partition lines instead
        # of 1024 separate 1KB lines.
        def dma_w_half(nm, wdram, h, eng):
            eng.dma_start(
                out=w_sb[nm][:, 4 * D * h:4 * D * (h + 1)],
                in_=wdram[:, 4 * D * h:4 * D * (h + 1)])

        def dma_x_half(xd, qu, xt, h, eng):
            eng.dma_start(
                out=xt[:, 4 * QW * h:4 * QW * (h + 1)],
                in_=xd[qu][:, 4 * QW * h:4 * QW * (h + 1)])

        # ---- projection building blocks ----
        # Per-queue DMA bandwidth is only ~130 GB/s, so spread the 1MB
        # quarter loads across three issue queues (scalar is reserved: an
        # exp stalled behind a DMA issue costs wall time directly).
        dma_flip = [0]
        _qrot = (nc.sync, nc.gpsimd)

        def dma_quarter(xd, qu, eng=None):
            xt = inx.tile([P, ET * QW], BF16, tag="inx", name="inx")
            if eng is None:
                eng = _qrot[dma_flip[0] % 2]
                dma_flip[0] += 1
            eng.dma_start(out=xt[:], in_=xd[qu])
            return xt

        def qk_group(nm, dst, xt, qu, i):
            """One psum group: qpT/kpT pd-tile i, token quarter qu.

            PSUM comes from the shared 1-bank "pv" tag so the 2-slot S
            rotation is never blocked behind a projection group."""
            ps = pv_pool.tile([P, QW], F32, tag="pv", name="ps")
            for e in range(ET):
                nc.tensor.matmul(
                    ps[:], w_sb[nm][:, D * e + P * i:D * e + P * (i + 1)],
                    xt[:, QW * e:QW * (e + 1)],
                    start=(e == 0), stop=(e == ET - 1))
            nc.vector.tensor_scalar(
                dst[i][:, QW * qu:QW * (qu + 1)], ps[:],
                bqk[nm][:, i:i + 1], None, Add)

        def v_group(xt, qu, tt_):
            t = 4 * qu + tt_
            ps = pv_pool.tile([P, D], F32, tag="pv", name="ps")
            for e in range(ET):
                nc.tensor.matmul(
                    ps[:], xt[:, QW * e + P * tt_:QW * e + P * (tt_ + 1)],
                    w_sb["v"][:, D * e:D * (e + 1)],
                    start=(e == 0), stop=(e == ET - 1))
            # bv is folded into bo on the host (softmax weights sum to 1,
            # so +bv per value adds exactly bv @ Wo to every token)
            nc.vector.tensor_copy(
                vpa[t].rearrange("p (h c) -> p h c", c=65)[:, :, 0:64],
                ps[:].rearrange("p (h c) -> p h c", c=64))

        # Interleaved projection task lists, one per early attention step.
        # Tokens: "D:nm:qu" = quarter DMA, "G:nm:qu:i" = qk group,
        # "V:qu:tt" = v group. Quarter DMAs sit ~6 task slots ahead of
        # their consumer group (a 1MB transfer takes ~7us on contended HBM).
        def make_tasks():
            # step 0 runs one task per granule (the quarter-1 qk groups
            # were pulled out of the pre-step into granules 0-1 so the
            # first exp only waits on the quarter-0 loads)
            t0 = ["G:q:1:0", "G:k:1:0",
                  "D:k:2", "D:v:0", "D:q:2", "D:v:1", "D:k:3",
                  "G:k:2:0", "V:0:0", "V:0:1", "G:q:2:0", "V:0:2",
                  "V:0:3", "D:q:3", "G:k:3:0", "V:1:0", "V:1:1",
                  "V:1:2", "D:v:2", "V:1:3", "G:q:3:0", "V:2:0",
                  "V:2:1", "D:v:3", "V:2:2", "V:2:3", "V:3:0",
                  "V:3:1", "V:3:2", "V:3:3"]
            tasks = {0: t0}
            for i in range(1, 4):
                tasks[i] = [
                    "D:k:0", "D:q:0", "D:k:1",
                    f"G:k:0:{i}", "D:q:1", f"G:q:0:{i}",
                    "D:k:2", f"G:k:1:{i}", "D:k:3", f"G:q:1:{i}",
                    f"G:k:2:{i}", f"G:k:3:{i}",
                ]
                tasks[i + 3] = [
                    "D:q:2", "D:q:3", f"G:q:2:{i}", f"G:q:3:{i}",
                ]
            return tasks

        proj_tasks = make_tasks()
        _src = {"q": qT, "k": kT, "v": vT}
        _dst = {"q": qpT, "k": kpT}

        def run_task(state, task):
            p = task.split(":")
            if p[0] == "D":
                state[(p[1], int(p[2]))] = dma_quarter(_src[p[1]], int(p[2]))
            elif p[0] == "G":
                nm, qu, i = p[1], int(p[2]), int(p[3])
                qk_group(nm, _dst[nm], state[(nm, qu)], qu, i)
            else:
                qu, tt_ = int(p[1]), int(p[2])
                v_group(state[("v", qu)], qu, tt_)

        # ---- pre-step: q & k pd-tile 0, token quarter 0 only (the
        # quarter-1 groups ride step-0 granules 0-1 as tasks).  The
        # first-exp critical set {wq, wk, q-q0, k-q0} = 4MB is balanced
        # across the three DMA queues (~107 B/ns each); wv rides gpsimd
        # right after so vpa[0] is ready for the first PV at G=LAG. ----
        xq = inx.tile([P, ET * QW], BF16, tag="inx", name="inx")
        xk = inx.tile([P, ET * QW], BF16, tag="inx", name="inx")
        dma_x_half(qT, 0, xq, 0, nc.scalar)
        dma_w_half("q", wq_d, 0, nc.sync)
        dma_w_half("k", wk_d, 0, nc.gpsimd)
        nc.sync.dma_start(out=bqk["q"][:], in_=bq_d.rearrange("i p -> p i"))
        nc.sync.dma_start(out=bqk["k"][:], in_=bk_d.rearrange("i p -> p i"))
        dma_x_half(qT, 0, xq, 1, nc.scalar)
        dma_w_half("q", wq_d, 1, nc.sync)
        dma_x_half(kT, 0, xk, 0, nc.gpsimd)
        dma_x_half(kT, 0, xk, 1, nc.sync)
        dma_w_half("k", wk_d, 1, nc.gpsimd)
        xq1 = dma_quarter(qT, 1, eng=nc.scalar)
        xk1 = dma_quarter(kT, 1, eng=nc.sync)
        nc.gpsimd.dma_start(out=w_sb["v"][:], in_=wv_d)
        dummy_mms(WARMUP_MMS)
        qk_group("q", qpT, xq, 0, 0)
        qk_group("k", kpT, xk, 0, 0)
        # hold wo + task quarters until the pre-step casts retire so they
        # don't steal HBM bandwidth from the critical-path startup loads
        nc.multi_engine_barrier(
            [mybir.EngineType.SP, mybir.EngineType.Pool,
             mybir.EngineType.DVE])
        nc.gpsimd.dma_start(out=wo_sb[:], in_=wo_d)

        # ---- output projection ----
        def outproj_tile(t, only_n=None):
            for n in ((0, 1) if only_n is None else (only_n,)):
                ps = pv_pool.tile([P, QW], F32, tag="pv", name="ps")
                for c in range(4):
                    nc.tensor.matmul(
                        ps[:], OT[c][:, P * t:P * (t + 1)],
                        wo_sb[:, E * c + QW * n:E * c + QW * (n + 1)],
                        start=(c == 0), stop=(c == 3))
                ost = ost_pool.tile([P, QW], BF16, tag="outst",
                                    name="outst")
                nc.vector.tensor_copy(ost[:], ps[:])
                eng = nc.sync if n == 0 else nc.gpsimd
                eng.dma_start(out=out[2 * t + n], in_=ost[:])

        # ---- attention machinery ----
        def s_granule(cur, saved_cur, gi):
            """S matmuls + exp for granule gi of step cur."""
            hp, blk = cur
            kt, j = gi // 2, gi % 2
            q0 = 1024 * blk + QW * j
            ps = ps_pool.tile([P, 1024], F32, tag="s", name="ps")
            for half in range(2):
                nc.tensor.matmul(
                    ps[:, QW * half:QW * (half + 1)],
                    kpT[hp][64 * half:64 * (half + 1), P * kt:P * (kt + 1)],
                    qpT[hp][64 * half:64 * (half + 1), q0:q0 + QW],
                    start=True, stop=True)
            e = expS.tile([P, 1024], BF16, tag="expS", name="expS")
            nc.scalar.activation(e[:], ps[:], Exp)
            saved_cur[kt][j] = e

        def pv_mms(prev, saved_prev, pv_live, key, jj, kk, nk=1):
            """PV (M=65: 64 V dims + ones col) for k-tiles kk..kk+nk-1."""
            php, pblk = prev
            if key not in pv_live:
                pv_live[key] = [
                    pv_pool.tile([65, QW], F32, tag="pv", name="pv")
                    for _ in range(2)]
            for k2 in range(kk, kk + nk):
                eS = saved_prev[k2][jj]
                for hh in range(2):
                    h = 2 * php + hh
                    nc.tensor.matmul(
                        pv_live[key][hh][:],
                        vpa[k2][:, 65 * h:65 * (h + 1)],
                        eS[:, QW * hh:QW * (hh + 1)],
                        start=(k2 == 0), stop=(k2 == TT - 1))

        def finish_pv_group(prev, pv_live, key, jj, den):
            """PV group done: stage unnormalized O^T + denominator rows."""
            php, pblk = prev
            qt = 2 * pblk + jj
            pvh = pv_live.pop(key)
            for hh in range(2):
                nc.vector.tensor_copy(
                    OT[php][64 * hh:64 * (hh + 1), QW * qt:QW * (qt + 1)],
                    pvh[hh][0:64, :])
                # custom-DVE recip can't route cross-partition reads:
                # stage the PSUM denominator row (partition 64) to an
                # SBUF partition-0 tile now, freeing the PSUM bank.
                st = scs_pool.tile([1, QW], F32, tag="scs", name="scs")
                nc.vector.tensor_copy(st[:], pvh[hh][64:65, :])
                den[2 * jj + hh] = st

        def norm_recip(pend, r):
            # in-place: custom-DVE ops require base partition 0, and the
            # staged [1,512] row is already there.
            st = pend["den"][r]
            nc.vector.reciprocal_approx_fast(st[:], st[:])

        def norm_apply(pend, r):
            jj, hh = r // 2, r % 2
            php, pblk = pend["step"]
            qt = 2 * pblk + jj
            bc = bc_pool.tile([P, QW], F32, tag="bc", name="bc")
            nc.gpsimd.partition_broadcast(bc[:], pend["den"][r][:],
                                          channels=P)
            sl = OT[php][64 * hh:64 * (hh + 1), QW * qt:QW * (qt + 1)]
            nc.vector.tensor_mul(sl, sl, bc[64 * hh:64 * (hh + 1), :])

        # ---- steady state: one global granule stream; PV trails S by
        # LAG granules (bounded by when wv + the first v quarter can land
        # under the startup DMA roofline).  Norms and outproj halves sit
        # at fixed global-granule offsets behind the denominators. ----
        NG = len(STEPS) * 2 * TT        # 256 S granules
        norm_sched = {}
        for pidx in range(len(STEPS)):
            G0 = 32 * pidx + LAG
            for r, (dg_r, dg_a) in enumerate(
                    ((17, 19), (18, 20), (33, 35), (34, 36))):
                norm_sched.setdefault(G0 + dg_r, []).append(("r", pidx, r))
                norm_sched.setdefault(G0 + dg_a, []).append(("a", pidx, r))

        def run_norms(G):
            for kind, pidx, r in norm_sched.pop(G, ()):
                if kind == "r":
                    norm_recip(pend[pidx], r)
                else:
                    norm_apply(pend[pidx], r)

        # outproj halves: tiles 0-3 once idx4's jj0 norms land (G>=165),
        # tiles 4-7 after its jj1 norms (G>=181); placed to stay disjoint
        # from the psum-using projection tasks of idx5/idx6 (granules
        # 16/24), keeping the "pv" tag at <= 4 live tiles.
        op_sched = {}
        for s in range(8):
            op_sched[166 + s] = (s // 2, s % 2)
            op_sched[190 + s] = (4 + s // 2, s % 2)

        saved = {}
        pv_live = {}
        dens = [dict() for _ in STEPS]
        pend = [{"step": STEPS[i], "den": dens[i]}
                for i in range(len(STEPS))]
        tasks, ntask, tstate = [], 0, {}
        for G in range(NG):
            idx, gi = divmod(G, 2 * TT)
            cur = STEPS[idx]
            if gi == 0:
                saved[cur] = [[None, None] for _ in range(TT)]
                tasks = proj_tasks.pop(idx, [])
                ntask = len(tasks)
                tstate = {("q", 1): xq1, ("k", 1): xk1} if idx == 0 else {}
            run_norms(G)
            pg = G - LAG
            if pg >= 0:
                pidx2, pgi = divmod(pg, 2 * TT)
                pstep = STEPS[pidx2]
                jj, kk = pgi // TT, pgi % TT
                pv_mms(pstep, saved[pstep], pv_live, (pidx2, jj), jj, kk)
                if kk == TT - 1:
                    finish_pv_group(pstep, pv_live, (pidx2, jj), jj,
                                    dens[pidx2])
                if pgi == 2 * TT - 1:
                    saved.pop(pstep)
            s_granule(cur, saved[cur], gi)
            if ntask:
                if idx == 0:
                    if gi < ntask:
                        run_task(tstate, tasks[gi])
                else:
                    t0_ = ntask * gi // (2 * TT)
                    t1_ = ntask * (gi + 1) // (2 * TT)
                    for ti in range(t0_, t1_):
                        run_task(tstate, tasks[ti])
            op = op_sched.pop(G, None)
            if op is not None:
                outproj_tile(op[0], only_n=op[1])

        # ---- drain: the trailing LAG granules of PV (2 k-tiles per
        # slot), then the idx7 norms and the last outproj tiles. ----
        for s in range(LAG // 2):
            run_norms(NG + s)
            pg = NG - LAG + 2 * s
            pidx2, pgi = divmod(pg, 2 * TT)
            pstep = STEPS[pidx2]
            jj, kk = pgi // TT, pgi % TT
            pv_mms(pstep, saved[pstep], pv_live, (pidx2, jj), jj, kk, nk=2)
            if kk + 1 == TT - 1:
                finish_pv_group(pstep, pv_live, (pidx2, jj), jj,
                                dens[pidx2])
        for t in range(8, 12):
            outproj_tile(t)
        for G in sorted(norm_sched):
            run_norms(G)
        for t in range(12, 16):
            outproj_tile(t)

def build_nc():
    nc = bacc.Bacc("TRN2", target_bir_lowering=False, debug=False,
                   num_devices=NCORES)
    # q/k/v ship as [quarter][partition][e-chunk*512t] and weights as
    # [partition][e-chunk*512d] ([p][c*1024e] for wo): SBUF-layout-exact,
    # so every load is one contiguous descriptor with 4-8KB lines.
    io = {
        "qT": nc.dram_tensor("qT", [4, P, ET * QW], BF16,
                             kind="ExternalInput").ap(),
        "kT": nc.dram_tensor("kT", [4, P, ET * QW], BF16,
                             kind="ExternalInput").ap(),
        "vT": nc.dram_tensor("vT", [4, P, ET * QW], BF16,
                             kind="ExternalInput").ap(),
        "wq": nc.dram_tensor("wq", [P, ET * D], BF16,
                             kind="ExternalInput").ap(),
        "wk": nc.dram_tensor("wk", [P, ET * D], BF16,
                             kind="ExternalInput").ap(),
        "wv": nc.dram_tensor("wv", [P, ET * D], BF16,
                             kind="ExternalInput").ap(),
        "wo": nc.dram_tensor("wo", [P, 4 * E], BF16,
                             kind="ExternalInput").ap(),
        "bq": nc.dram_tensor("bq", [PT, P], F32, kind="ExternalInput").ap(),
        "bk": nc.dram_tensor("bk", [PT, P], F32, kind="ExternalInput").ap(),
        # one contiguous [128, 512] bf16 block per outproj half
        "out": nc.dram_tensor("out", [2 * TT, P, QW], BF16,
                              kind="ExternalOutput").ap(),
    }
    with tile.TileContext(nc) as tc:
        _emit(nc, tc, io)
    nc.compile()
    return nc


_NC = None


def _get_nc():
    global _NC
    if _NC is None:
        _NC = build_nc()
    return _NC


def _x4(xb):
    """[L, E] batch slice -> [quarter, p, e*512t] device layout (bf16)."""
    # element (qu, p, e, t) = x.T[128e + p, 512qu + t]
    a = xb.T.reshape(ET, P, 4, QW).transpose(2, 1, 0, 3)
    return np.ascontiguousarray(a.reshape(4, P, ET * QW)).astype(BF16_NP)


def _w4(w):
    """[E, D] weight slice -> [p, e*512d] device layout (bf16)."""
    a = w.reshape(ET, P, D).transpose(1, 0, 2)
    return np.ascontiguousarray(a.reshape(P, ET * D)).astype(BF16_NP)


def make_in_maps(q, k, v, Wq, bq, Wk, bk, Wv, Wo):
    scale = np.float32(1.0 / math.sqrt(HD))
    in_maps = []
    for c in range(NCORES):
        b, g = divmod(c, 2)
        sl = slice(g * D, (g + 1) * D)
        wo4 = Wo[sl, :].reshape(4, P, E).transpose(1, 0, 2)
        in_maps.append({
            "qT": _x4(q[b]),
            "kT": _x4(k[b]),
            "vT": _x4(v[b]),
            "wq": _w4(Wq[:, sl] * scale),
            "wk": _w4(Wk[:, sl]),
            "wv": _w4(Wv[:, sl]),
            "wo": np.ascontiguousarray(
                wo4.reshape(P, 4 * E)).astype(BF16_NP),
            "bq": (bq[sl] * scale).reshape(PT, P).astype(np.float32),
            "bk": bk[sl].reshape(PT, P).astype(np.float32),
        })
    return in_maps


def kernel(q, k, v, mask, Wq, bq, Wk, bk, Wv, bv, Wo, bo):
    global LAST_EXEC_NS, LAST_RESULTS
    q, k, v = (np.asarray(x, np.float32) for x in (q, k, v))
    Wq, bq, Wk, bk, Wv, bv, Wo, bo = (
        np.asarray(x, np.float32)
        for x in (Wq, bq, Wk, bk, Wv, bv, Wo, bo))
    nc = _get_nc()
    in_maps = make_in_maps(q, k, v, Wq, bq, Wk, bk, Wv, Wo)
    kwargs = {}
    if TRACE:
        kwargs = dict(trace=True)
    res = run_bass_kernel_spmd(nc, in_maps, list(range(NCORES)), **kwargs)
    LAST_EXEC_NS = res.exec_time_ns
    LAST_RESULTS = res
    outs = []
    for c in range(NCORES):
        o = np.asarray(res.results[c]["out"], np.float32)
        # [t*2+n, p, 512] -> [L, E]
        outs.append(o.reshape(TT, 2, P, QW).transpose(0, 2, 1, 3)
                    .reshape(L, E))
    full = np.stack([outs[2 * b] + outs[2 * b + 1] for b in range(B)], axis=0)
    # bv folded: softmax rows sum to 1, so +bv on v adds bv @ Wo to
    # every token
    full += (bo + bv @ Wo)[None, None, :].astype(np.float32)
    return full.astype(np.float32)

